# revision 35
# baseline (speedup 1.0000x reference)
"""Trainium2 Bass kernel for complex Chebyshev graph conv with attention.

Problem shapes (hardcoded):
  B=4, N=512, C_IN=32, K+1=4 poly terms, H=4 heads, P=64 out/head, ACT=256.

v2 design:
  Sharding: core = 2*b + jh  (jh = j-half). Each core handles ALL 4 heads,
  ALL k, for j in [jh*256, jh*256+256) of graph b — so each core streams
  only HALF of that graph's L (bf16), and the softmax normalization is done
  on the host: Y = (Y_coreA + Y_coreB) / (den_A + den_B).

  Weight folding: host precomputes A[k,h] = X @ W[k,h] (complex, [N,64]).
  With lhsT variants [Ar|Ai] and [-Ai|Ar], the propagation matmuls
  accumulate Yre (psum rows 0:64) and Yim (rows 64:128) directly across
  all (k, jc) — no LX intermediate, no weight matmuls, no sub/add fixups.

  Score path (per head h, j-chunk jc): pre = prelu(si[i] + sj[j]) via ACT
  (si broadcast across partitions via a PE ones-matmul, sj as per-partition
  bias); sq = pre^2; ssum = sq_re + sq_im; E = exp(ssum) in bf16.
  den[h,i] = sum_j E via ones-matmul (partial; host combines).

  Propagation: Mcat[k,ri] = Lt[k,ri] * E (one DVE op per (h,jc) with a
  stride-0-replicated E access pattern), then 8 accumulating matmuls.
"""

import numpy as np

B, N, C = 4, 512, 32
K1, H, P = 4, 4, 64
ACT_OUT = P * H
JH = 256   # j rows per core (half a graph)
NJC = 2    # j chunks of 128 per core

_cache = {}


def _build_bass(alpha_re=0.25, alpha_im=0.25):
    import concourse.bass as bass
    import concourse.mybir as mybir
    import concourse.tile as tile
    from concourse import bacc

    fp32 = mybir.dt.float32
    f32r = mybir.dt.float32r
    bf16 = mybir.dt.bfloat16
    AF = mybir.ActivationFunctionType

    nc = bacc.Bacc("TRN2", target_bir_lowering=False, debug=False)

    ltc = nc.declare_dram_parameter("ltc", [JH, 2 * K1, N], bf16, isOutput=False)
    acat = nc.declare_dram_parameter("acat", [H, JH, K1, 2, 128], bf16,
                                     isOutput=False)
    siw = nc.declare_dram_parameter("siw", [1, H, 2 * N], f32r, isOutput=False)
    sjc = nc.declare_dram_parameter("sjc", [128, NJC, H, 2], fp32, isOutput=False)
    ones_r = nc.declare_dram_parameter("ones_r", [128], f32r, isOutput=False)
    ones_c = nc.declare_dram_parameter("ones_c", [128], bf16, isOutput=False)
    yout = nc.declare_dram_parameter("yout", [H, 128, N], fp32, isOutput=True)
    dout = nc.declare_dram_parameter("dout", [1, H * N], fp32, isOutput=True)

    def rep(sl, n):
        # replicate a [128, F] slice n times along a new middle free dim
        ap = list(sl.ap)
        return bass.AP(tensor=sl.tensor, offset=sl.offset,
                       ap=[ap[0], [0, n]] + ap[1:])

    with tile.TileContext(nc) as tc, nc.allow_low_precision(
            reason="bf16 propagation operands (full-rate PE, 2x DVE)"):
        consts = tc.alloc_tile_pool(name="consts", bufs=1)
        ep = tc.alloc_tile_pool(name="ep", bufs=3)
        sc = tc.alloc_tile_pool(name="sc", bufs=4)
        mp = tc.alloc_tile_pool(name="mp", bufs=3)
        yp = tc.alloc_tile_pool(name="yp", bufs=2)
        pbsi = tc.alloc_tile_pool(name="pbsi", bufs=2, space="PSUM")
        py = tc.alloc_tile_pool(name="py", bufs=2, space="PSUM")
        pden = tc.alloc_tile_pool(name="pden", bufs=2, space="PSUM")
        pools = [consts, ep, sc, mp, yp, pbsi, py, pden]

        # warm the ACT tables needed later so table loads don't land
        # mid-pipeline (memset on Pool: keep DVE free for the small DMAs)
        warm = consts.tile([1, 4], fp32)
        nc.gpsimd.memset(warm, 1.0)
        nc.scalar.activation(warm, warm, AF.Prelu, alpha=alpha_re)
        nc.scalar.activation(warm, warm, AF.Square)
        nc.scalar.activation(warm, warm, AF.Exp)

        # ---- DMA order matters: transfers serialize globally, so the tiny
        # score-path inputs go FIRST on SP, then L/A streams in first-use
        # order. ones_r rides the ACT queue (post-warm), ones_c the SWDGE. ----
        ones_row = consts.tile([1, 128], f32r)
        nc.sync.dma_start(out=ones_row, in_=ones_r[:].rearrange("(o n) -> o n", o=1))
        si_sb = consts.tile([1, H, 2 * N], f32r)
        nc.sync.dma_start(out=si_sb, in_=siw[:])
        sjc_sb = consts.tile([128, NJC, H, 2], fp32)
        nc.sync.dma_start(out=sjc_sb, in_=sjc[:])
        ones_col = consts.tile([128, 1], bf16)
        nc.gpsimd.dma_start(out=ones_col, in_=ones_c[:].rearrange("(n o) -> n o", o=1))

        ltc_sb = consts.tile([128, NJC, 2 * K1, N], bf16)
        acat_sb = consts.tile([128, H, NJC, K1, 2, 128], bf16)

        def acat_dma(h):
            nc.sync.dma_start(
                out=acat_sb[:, h],
                in_=acat[h].rearrange("(c p) k v o -> p c (k v o)", p=128))

        nc.sync.dma_start(out=ltc_sb[:, 0, 0:4, :], in_=ltc[0:128, 0:4, :])
        acat_dma(0)
        nc.sync.dma_start(out=ltc_sb[:, 0, 4:8, :], in_=ltc[0:128, 4:8, :])
        nc.sync.dma_start(out=ltc_sb[:, 1, 0:4, :], in_=ltc[128:256, 0:4, :])
        nc.sync.dma_start(out=ltc_sb[:, 1, 4:8, :], in_=ltc[128:256, 4:8, :])
        acat_dma(1)
        acat_dma(2)
        acat_dma(3)

        E = [None] * H
        bsi = [None] * H
        den_sb = consts.tile([1, H * N], fp32)

        def emit_bsi(h):
            # si broadcast across partitions via PE; prelu reads it from PSUM
            bps = pbsi.tile([128, 2 * N], fp32, tag="bsi")
            bsi[h] = bps
            nc.tensor.matmul(bps[:, 0:N], ones_row, si_sb[:, h, 0:N],
                             start=True, stop=True)
            nc.tensor.matmul(bps[:, N:2 * N], ones_row, si_sb[:, h, N:2 * N],
                             start=True, stop=True)

        f16 = mybir.dt.float16

        def emit_score_front(h, fast=False):
            # prelus + squares in fp16 (2x DVE on the square; exp's input
            # error ~1e-3 relative stays well under the gate)
            E[h] = ep.tile([128, NJC, N], bf16, tag="E", name=f"E{h}")
            pres = []
            for jc in range(NJC):
                pre = sc.tile([128, 2 * N], f16, tag="pre")
                nc.scalar.activation(pre[:, 0:N], bsi[h][:, 0:N], AF.Prelu,
                                     bias=sjc_sb[:, jc, h, 0:1], alpha=alpha_re)
                nc.scalar.activation(pre[:, N:2 * N], bsi[h][:, N:2 * N],
                                     AF.Prelu, bias=sjc_sb[:, jc, h, 1:2],
                                     alpha=alpha_im)
                pres.append(pre)
            sq0 = sc.tile([128, 2 * N], f16, tag="sq0")
            if h < 2:
                nc.scalar.activation(sq0, pres[0], AF.Square)
            else:
                nc.gpsimd.tensor_mul(sq0, pres[0], pres[0])
            sq1 = sc.tile([128, 2 * N], f16, tag="sq1")
            nc.vector.tensor_mul(sq1, pres[1], pres[1])
            return [sq0, sq1]

        def emit_score_back(h, sqs, fast=False):
            veng = nc.vector if fast else nc.gpsimd
            for jc in range(NJC):
                ssum = sc.tile([128, N], f16, tag="ssum")
                veng.tensor_add(ssum, sqs[jc][:, 0:N], sqs[jc][:, N:2 * N])
                nc.scalar.activation(E[h][:, jc, :], ssum, AF.Exp)

        def emit_den(h):
            dps = pden.tile([1, N], fp32, tag="den")
            for jc in range(NJC):
                nc.tensor.matmul(dps, ones_col, E[h][:, jc, :],
                                 start=(jc == 0), stop=(jc == NJC - 1))
            if h % 2 == 0:
                nc.vector.tensor_copy(den_sb[:, h * N:(h + 1) * N], dps)
            else:
                nc.scalar.copy(den_sb[:, h * N:(h + 1) * N], dps)

        ps_y = [None] * H

        def emit_kloop(h, pool_share=True):
            ps = py.tile([128, N], fp32, tag="ps_y")
            ps_y[h] = ps
            for jc in range(NJC):
                mcat = mp.tile([128, 2 * K1, N], bf16, tag="mcat")
                if pool_share and jc == 1:
                    # Pool takes the k3 pair of jc1; DVE the rest (split on
                    # the last head so its matmuls can start sooner)
                    if h == H - 1:
                        nc.vector.tensor_mul(mcat[:, 0:4, :],
                                             ltc_sb[:, jc, 0:4, :],
                                             rep(E[h][:, jc, :], 4))
                        nc.vector.tensor_mul(mcat[:, 4:6, :],
                                             ltc_sb[:, jc, 4:6, :],
                                             rep(E[h][:, jc, :], 2))
                    else:
                        nc.vector.tensor_mul(mcat[:, 0:6, :],
                                             ltc_sb[:, jc, 0:6, :],
                                             rep(E[h][:, jc, :], 6))
                    nc.gpsimd.tensor_mul(mcat[:, 6:8, :], ltc_sb[:, jc, 6:8, :],
                                         rep(E[h][:, jc, :], 2))
                else:
                    nc.vector.tensor_mul(mcat, ltc_sb[:, jc],
                                         rep(E[h][:, jc, :], 2 * K1))
                for k in range(K1):
                    nc.tensor.matmul(ps, acat_sb[:, h, jc, k, 0],
                                     mcat[:, 2 * k, :],
                                     start=(jc == 0 and k == 0), stop=False)
                    nc.tensor.matmul(ps, acat_sb[:, h, jc, k, 1],
                                     mcat[:, 2 * k + 1, :],
                                     start=False,
                                     stop=(jc == NJC - 1 and k == K1 - 1))

        dmaq = [nc.sync, nc.scalar, nc.scalar, nc.sync]

        def emit_out(h):
            y_sb = yp.tile([128, N], fp32, tag="y_sb")
            if h == H - 1:
                # last head: split the copy ACT/DVE so the tail is short
                nc.scalar.copy(y_sb[:, 0:N // 2], ps_y[h][:, 0:N // 2])
                nc.vector.tensor_copy(y_sb[:, N // 2:N], ps_y[h][:, N // 2:N])
            elif h % 2 == 0:
                nc.scalar.copy(y_sb, ps_y[h])
            else:
                nc.vector.tensor_copy(y_sb, ps_y[h])
            dmaq[h].dma_start(out=yout[h], in_=y_sb)

        # ---- pipelined emission: 2-deep score lookahead ----
        emit_bsi(0)
        sq_0 = emit_score_front(0, fast=True)
        emit_score_back(0, sq_0, fast=True)
        emit_bsi(1)
        sq_1 = emit_score_front(1)
        emit_score_back(1, sq_1)

        sqs = {}
        for h in range(H):
            if h + 2 < H:
                emit_bsi(h + 2)
                sqs[h + 2] = emit_score_front(h + 2)
            emit_den(h)
            emit_kloop(h, pool_share=(h > 0))
            if h + 2 < H:
                emit_score_back(h + 2, sqs[h + 2])
            emit_out(h)
        nc.sync.dma_start(out=dout[:], in_=den_sb)

        for p_ in reversed(pools):
            p_.release()

    nc.compile()
    return nc


def _host_prep(inputs):
    """Build the 8 per-core input maps from the full inputs."""
    import ml_dtypes
    bf = ml_dtypes.bfloat16
    Xr = np.asarray(inputs["X_real"], np.float32)
    Xi = np.asarray(inputs["X_imag"], np.float32)
    Lr = np.asarray(inputs["L_real"], np.float32)
    Li = np.asarray(inputs["L_imag"], np.float32)
    awr = np.asarray(inputs["attn_w_real"], np.float32)
    awi = np.asarray(inputs["attn_w_imag"], np.float32)
    abr = np.asarray(inputs["attn_b_real"], np.float32)
    abi = np.asarray(inputs["attn_b_imag"], np.float32)
    wr = np.asarray(inputs["weight_real"], np.float32)
    wi = np.asarray(inputs["weight_imag"], np.float32)

    W1r, W2r = awr[:C], awr[C:]
    W1i, W2i = awi[:C], awi[C:]
    si_re = Xr @ W1r - Xi @ W1i + abr  # (B,N,H) (+bias folded)
    si_im = Xr @ W1i + Xi @ W1r + abi
    sj_re = Xr @ W2r - Xi @ W2i
    sj_im = Xr @ W2i + Xi @ W2r

    LTr = Lr.swapaxes(-1, -2)  # (B,K1,j,i)
    LTi = Li.swapaxes(-1, -2)

    # A[b,k,j,p,h] = complex X @ W per head
    Wr4 = wr.reshape(K1, C, P, H)
    Wi4 = wi.reshape(K1, C, P, H)
    Ar = (np.einsum('bjc,kcph->bkjph', Xr, Wr4)
          - np.einsum('bjc,kcph->bkjph', Xi, Wi4))
    Ai = (np.einsum('bjc,kcph->bkjph', Xi, Wr4)
          + np.einsum('bjc,kcph->bkjph', Xr, Wi4))

    in_maps = []
    for core in range(8):
        b, jh = core // 2, core % 2
        js = slice(jh * JH, (jh + 1) * JH)
        ltc = np.empty((JH, 2 * K1, N), np.float32)
        ltc[:, 0::2, :] = LTr[b, :, js, :].swapaxes(0, 1)
        ltc[:, 1::2, :] = LTi[b, :, js, :].swapaxes(0, 1)
        # acat[h,j,k,0] = [Ar|Ai], acat[h,j,k,1] = [-Ai|Ar]
        acat = np.empty((H, JH, K1, 2, 128), np.float32)
        arh = Ar[b, :, js].transpose(3, 1, 0, 2)  # (h,j,k,p)
        aih = Ai[b, :, js].transpose(3, 1, 0, 2)
        acat[..., 0, 0:P] = arh
        acat[..., 0, P:128] = aih
        acat[..., 1, 0:P] = -aih
        acat[..., 1, P:128] = arh
        siw = np.empty((1, H, 2 * N), np.float32)
        siw[0, :, 0:N] = si_re[b].T
        siw[0, :, N:2 * N] = si_im[b].T
        sjc = np.empty((128, NJC, H, 2), np.float32)
        for jc in range(NJC):
            rows = slice(jh * JH + jc * 128, jh * JH + (jc + 1) * 128)
            sjc[:, jc, :, 0] = sj_re[b, rows, :]
            sjc[:, jc, :, 1] = sj_im[b, rows, :]
        in_maps.append({
            "ltc": ltc.astype(bf),
            "acat": np.ascontiguousarray(acat).astype(bf),
            "siw": siw,
            "sjc": sjc,
            "ones_r": np.ones(128, np.float32),
            "ones_c": np.ones(128, bf),
        })
    return in_maps


def _host_post(results, inputs):
    br = np.asarray(inputs["bias_real"], np.float32)
    bi = np.asarray(inputs["bias_imag"], np.float32)
    out_re = np.empty((B, N, P, H), np.float32)
    out_im = np.empty((B, N, P, H), np.float32)
    for b in range(B):
        y = results[2 * b]["yout"] + results[2 * b + 1]["yout"]  # (H,128,N)
        den = (results[2 * b]["dout"] + results[2 * b + 1]["dout"]).reshape(H, N)
        for h in range(H):
            out_re[b, :, :, h] = (y[h, 0:P] / den[h]).T
            out_im[b, :, :, h] = (y[h, P:128] / den[h]).T
    out_re = out_re.reshape(B, N, ACT_OUT) + br
    out_im = out_im.reshape(B, N, ACT_OUT) + bi
    return out_re, out_im


def _run(inputs, trace=False, **kw):
    from concourse.bass_utils import run_bass_kernel_spmd
    a_re = float(np.asarray(inputs["prelu_a_real"]))
    a_im = float(np.asarray(inputs["prelu_a_imag"]))
    key = ("nc", a_re, a_im)
    if key not in _cache:
        _cache[key] = _build_bass(a_re, a_im)
    nc = _cache[key]
    _cache["nc"] = nc  # for sim_time/trace_tool
    in_maps = _host_prep(inputs)
    res = run_bass_kernel_spmd(nc, in_maps, list(range(8)), trace=trace, **kw)
    out = _host_post(res.results, inputs)
    return out, res


def kernel(**inputs):
    out, _ = _run(inputs, trace=False)
    return out


# revision 36
# speedup vs baseline: 1.0807x; 1.0807x over previous
"""Trainium2 Bass kernel for complex Chebyshev graph conv with attention.

Problem shapes (hardcoded):
  B=4, N=512, C_IN=32, K+1=4 poly terms, H=4 heads, P=64 out/head, ACT=256.

v2 design:
  Sharding: core = 2*b + jh  (jh = j-half). Each core handles ALL 4 heads,
  ALL k, for j in [jh*256, jh*256+256) of graph b — so each core streams
  only HALF of that graph's L (bf16), and the softmax normalization is done
  on the host: Y = (Y_coreA + Y_coreB) / (den_A + den_B).

  Weight folding: host precomputes A[k,h] = X @ W[k,h] (complex, [N,64]).
  With lhsT variants [Ar|Ai] and [-Ai|Ar], the propagation matmuls
  accumulate Yre (psum rows 0:64) and Yim (rows 64:128) directly across
  all (k, jc) — no LX intermediate, no weight matmuls, no sub/add fixups.

  Score path (per head h, j-chunk jc): pre = prelu(si[i] + sj[j]) via ACT
  (si broadcast across partitions via a PE ones-matmul, sj as per-partition
  bias); sq = pre^2; ssum = sq_re + sq_im; E = exp(ssum) in bf16.
  den[h,i] = sum_j E via ones-matmul (partial; host combines).

  Propagation: Mcat[k,ri] = Lt[k,ri] * E (one DVE op per (h,jc) with a
  stride-0-replicated E access pattern), then 8 accumulating matmuls.
"""

import numpy as np

B, N, C = 4, 512, 32
K1, H, P = 4, 4, 64
ACT_OUT = P * H
JH = 256   # j rows per core (half a graph)
NJC = 2    # j chunks of 128 per core

_cache = {}


def _build_bass(alpha_re=0.25, alpha_im=0.25):
    import concourse.bass as bass
    import concourse.mybir as mybir
    import concourse.tile as tile
    from concourse import bacc

    fp32 = mybir.dt.float32
    f32r = mybir.dt.float32r
    bf16 = mybir.dt.bfloat16
    AF = mybir.ActivationFunctionType

    nc = bacc.Bacc("TRN2", target_bir_lowering=False, debug=False)

    ltc = nc.declare_dram_parameter("ltc", [JH, 2 * K1, N], bf16, isOutput=False)
    acat = nc.declare_dram_parameter("acat", [H, JH, K1, 2, 128], bf16,
                                     isOutput=False)
    siw = nc.declare_dram_parameter("siw", [1, H, 2 * N], f32r, isOutput=False)
    sjc = nc.declare_dram_parameter("sjc", [128, NJC, H, 2], fp32, isOutput=False)
    ones_r = nc.declare_dram_parameter("ones_r", [128], f32r, isOutput=False)
    ones_c = nc.declare_dram_parameter("ones_c", [128], bf16, isOutput=False)
    yout = nc.declare_dram_parameter("yout", [H, 128, N], fp32, isOutput=True)
    dout = nc.declare_dram_parameter("dout", [1, H * N], fp32, isOutput=True)

    def rep(sl, n):
        # replicate a [128, F] slice n times along a new middle free dim
        ap = list(sl.ap)
        return bass.AP(tensor=sl.tensor, offset=sl.offset,
                       ap=[ap[0], [0, n]] + ap[1:])

    with tile.TileContext(nc) as tc, nc.allow_low_precision(
            reason="bf16 propagation operands (full-rate PE, 2x DVE)"):
        consts = tc.alloc_tile_pool(name="consts", bufs=1)
        ep = tc.alloc_tile_pool(name="ep", bufs=3)
        sc = tc.alloc_tile_pool(name="sc", bufs=4)
        mp = tc.alloc_tile_pool(name="mp", bufs=3)
        yp = tc.alloc_tile_pool(name="yp", bufs=2)
        pbsi = tc.alloc_tile_pool(name="pbsi", bufs=2, space="PSUM")
        py = tc.alloc_tile_pool(name="py", bufs=2, space="PSUM")
        pden = tc.alloc_tile_pool(name="pden", bufs=2, space="PSUM")
        pools = [consts, ep, sc, mp, yp, pbsi, py, pden]

        # warm the ACT tables needed later so table loads don't land
        # mid-pipeline (memset on Pool: keep DVE free for the small DMAs)
        warm = consts.tile([1, 4], fp32)
        nc.gpsimd.memset(warm, 1.0)
        nc.scalar.activation(warm, warm, AF.Prelu, alpha=alpha_re)
        nc.scalar.activation(warm, warm, AF.Square)
        nc.scalar.activation(warm, warm, AF.Exp)

        # ---- DMA order matters: transfers serialize globally, so the tiny
        # score-path inputs go FIRST on SP, then L/A streams in first-use
        # order. ones_r rides the ACT queue (post-warm), ones_c the SWDGE. ----
        ones_row = consts.tile([1, 128], f32r)
        nc.sync.dma_start(out=ones_row, in_=ones_r[:].rearrange("(o n) -> o n", o=1))
        si_sb = consts.tile([1, H, 2 * N], f32r)
        nc.sync.dma_start(out=si_sb, in_=siw[:])
        sjc_sb = consts.tile([128, NJC, H, 2], fp32)
        nc.sync.dma_start(out=sjc_sb, in_=sjc[:])
        ones_col = consts.tile([128, 1], bf16)
        nc.gpsimd.dma_start(out=ones_col, in_=ones_c[:].rearrange("(n o) -> n o", o=1))

        ltc_sb = consts.tile([128, NJC, 2 * K1, N], bf16)
        acat_sb = consts.tile([128, H, NJC, K1, 2, 128], bf16)

        def acat_dma(h):
            nc.sync.dma_start(
                out=acat_sb[:, h],
                in_=acat[h].rearrange("(c p) k v o -> p c (k v o)", p=128))

        nc.sync.dma_start(out=ltc_sb[:, 0, 0:4, :], in_=ltc[0:128, 0:4, :])
        acat_dma(0)
        nc.sync.dma_start(out=ltc_sb[:, 0, 4:8, :], in_=ltc[0:128, 4:8, :])
        nc.sync.dma_start(out=ltc_sb[:, 1, 0:4, :], in_=ltc[128:256, 0:4, :])
        nc.sync.dma_start(out=ltc_sb[:, 1, 4:8, :], in_=ltc[128:256, 4:8, :])
        acat_dma(1)
        acat_dma(2)
        acat_dma(3)

        E = [None] * H
        bsi = [None] * H
        den_sb = consts.tile([1, H * N], fp32)

        def emit_bsi(h):
            # si broadcast across partitions via PE; prelu reads it from PSUM
            bps = pbsi.tile([128, 2 * N], fp32, tag="bsi")
            bsi[h] = bps
            nc.tensor.matmul(bps[:, 0:N], ones_row, si_sb[:, h, 0:N],
                             start=True, stop=True)
            nc.tensor.matmul(bps[:, N:2 * N], ones_row, si_sb[:, h, N:2 * N],
                             start=True, stop=True)

        f16 = mybir.dt.float16

        def emit_score_front(h, fast=False):
            # prelus + squares in fp16 (2x DVE on the square; exp's input
            # error ~1e-3 relative stays well under the gate)
            E[h] = ep.tile([128, NJC, N], bf16, tag="E", name=f"E{h}")
            pres = []
            for jc in range(NJC):
                pre = sc.tile([128, 2 * N], f16, tag="pre")
                nc.scalar.activation(pre[:, 0:N], bsi[h][:, 0:N], AF.Prelu,
                                     bias=sjc_sb[:, jc, h, 0:1], alpha=alpha_re)
                nc.scalar.activation(pre[:, N:2 * N], bsi[h][:, N:2 * N],
                                     AF.Prelu, bias=sjc_sb[:, jc, h, 1:2],
                                     alpha=alpha_im)
                pres.append(pre)
            sq0 = sc.tile([128, 2 * N], f16, tag="sq0")
            nc.scalar.activation(sq0, pres[0], AF.Square)
            sq1 = sc.tile([128, 2 * N], f16, tag="sq1")
            nc.vector.tensor_mul(sq1, pres[1], pres[1])
            return [sq0, sq1]

        def emit_score_back(h, sqs, fast=False):
            veng = nc.vector if fast else nc.gpsimd
            for jc in range(NJC):
                ssum = sc.tile([128, N], f16, tag="ssum")
                veng.tensor_add(ssum, sqs[jc][:, 0:N], sqs[jc][:, N:2 * N])
                nc.scalar.activation(E[h][:, jc, :], ssum, AF.Exp)

        def emit_den(h):
            dps = pden.tile([1, N], fp32, tag="den")
            for jc in range(NJC):
                nc.tensor.matmul(dps, ones_col, E[h][:, jc, :],
                                 start=(jc == 0), stop=(jc == NJC - 1))
            if h % 2 == 0:
                nc.vector.tensor_copy(den_sb[:, h * N:(h + 1) * N], dps)
            else:
                nc.scalar.copy(den_sb[:, h * N:(h + 1) * N], dps)

        ps_y = [None] * H

        def emit_kloop(h, pool_share=True):
            ps = py.tile([128, N], fp32, tag="ps_y")
            ps_y[h] = ps
            for jc in range(NJC):
                mcat = mp.tile([128, 2 * K1, N], bf16, tag="mcat")
                if pool_share and jc == 1:
                    # Pool takes the k3 pair of jc1; DVE the rest (split on
                    # the last head so its matmuls can start sooner)
                    if h == H - 1:
                        nc.vector.tensor_mul(mcat[:, 0:4, :],
                                             ltc_sb[:, jc, 0:4, :],
                                             rep(E[h][:, jc, :], 4))
                        nc.vector.tensor_mul(mcat[:, 4:6, :],
                                             ltc_sb[:, jc, 4:6, :],
                                             rep(E[h][:, jc, :], 2))
                    else:
                        nc.vector.tensor_mul(mcat[:, 0:6, :],
                                             ltc_sb[:, jc, 0:6, :],
                                             rep(E[h][:, jc, :], 6))
                    nc.gpsimd.tensor_mul(mcat[:, 6:8, :], ltc_sb[:, jc, 6:8, :],
                                         rep(E[h][:, jc, :], 2))
                else:
                    nc.vector.tensor_mul(mcat, ltc_sb[:, jc],
                                         rep(E[h][:, jc, :], 2 * K1))
                for k in range(K1):
                    nc.tensor.matmul(ps, acat_sb[:, h, jc, k, 0],
                                     mcat[:, 2 * k, :],
                                     start=(jc == 0 and k == 0), stop=False)
                    nc.tensor.matmul(ps, acat_sb[:, h, jc, k, 1],
                                     mcat[:, 2 * k + 1, :],
                                     start=False,
                                     stop=(jc == NJC - 1 and k == K1 - 1))

        dmaq = [nc.sync, nc.scalar, nc.scalar, nc.sync]

        def emit_out(h):
            y_sb = yp.tile([128, N], fp32, tag="y_sb")
            if h == H - 1:
                # last head: split the copy ACT/DVE so the tail is short
                nc.scalar.copy(y_sb[:, 0:N // 2], ps_y[h][:, 0:N // 2])
                nc.vector.tensor_copy(y_sb[:, N // 2:N], ps_y[h][:, N // 2:N])
            elif h % 2 == 0:
                nc.scalar.copy(y_sb, ps_y[h])
            else:
                nc.vector.tensor_copy(y_sb, ps_y[h])
            dmaq[h].dma_start(out=yout[h], in_=y_sb)

        # ---- pipelined emission: 2-deep score lookahead ----
        emit_bsi(0)
        sq_0 = emit_score_front(0, fast=True)
        emit_score_back(0, sq_0, fast=True)
        emit_bsi(1)
        sq_1 = emit_score_front(1)
        emit_score_back(1, sq_1)

        sqs = {}
        for h in range(H):
            if h + 2 < H:
                emit_bsi(h + 2)
                sqs[h + 2] = emit_score_front(h + 2)
            emit_den(h)
            emit_kloop(h, pool_share=(h > 0))
            if h + 2 < H:
                emit_score_back(h + 2, sqs[h + 2])
            emit_out(h)
        nc.sync.dma_start(out=dout[:], in_=den_sb)

        for p_ in reversed(pools):
            p_.release()

    nc.compile()
    return nc


def _host_prep(inputs):
    """Build the 8 per-core input maps from the full inputs."""
    import ml_dtypes
    bf = ml_dtypes.bfloat16
    Xr = np.asarray(inputs["X_real"], np.float32)
    Xi = np.asarray(inputs["X_imag"], np.float32)
    Lr = np.asarray(inputs["L_real"], np.float32)
    Li = np.asarray(inputs["L_imag"], np.float32)
    awr = np.asarray(inputs["attn_w_real"], np.float32)
    awi = np.asarray(inputs["attn_w_imag"], np.float32)
    abr = np.asarray(inputs["attn_b_real"], np.float32)
    abi = np.asarray(inputs["attn_b_imag"], np.float32)
    wr = np.asarray(inputs["weight_real"], np.float32)
    wi = np.asarray(inputs["weight_imag"], np.float32)

    W1r, W2r = awr[:C], awr[C:]
    W1i, W2i = awi[:C], awi[C:]
    si_re = Xr @ W1r - Xi @ W1i + abr  # (B,N,H) (+bias folded)
    si_im = Xr @ W1i + Xi @ W1r + abi
    sj_re = Xr @ W2r - Xi @ W2i
    sj_im = Xr @ W2i + Xi @ W2r

    LTr = Lr.swapaxes(-1, -2)  # (B,K1,j,i)
    LTi = Li.swapaxes(-1, -2)

    # A[b,k,j,p,h] = complex X @ W per head
    Wr4 = wr.reshape(K1, C, P, H)
    Wi4 = wi.reshape(K1, C, P, H)
    Ar = (np.einsum('bjc,kcph->bkjph', Xr, Wr4)
          - np.einsum('bjc,kcph->bkjph', Xi, Wi4))
    Ai = (np.einsum('bjc,kcph->bkjph', Xi, Wr4)
          + np.einsum('bjc,kcph->bkjph', Xr, Wi4))

    in_maps = []
    for core in range(8):
        b, jh = core // 2, core % 2
        js = slice(jh * JH, (jh + 1) * JH)
        ltc = np.empty((JH, 2 * K1, N), np.float32)
        ltc[:, 0::2, :] = LTr[b, :, js, :].swapaxes(0, 1)
        ltc[:, 1::2, :] = LTi[b, :, js, :].swapaxes(0, 1)
        # acat[h,j,k,0] = [Ar|Ai], acat[h,j,k,1] = [-Ai|Ar]
        acat = np.empty((H, JH, K1, 2, 128), np.float32)
        arh = Ar[b, :, js].transpose(3, 1, 0, 2)  # (h,j,k,p)
        aih = Ai[b, :, js].transpose(3, 1, 0, 2)
        acat[..., 0, 0:P] = arh
        acat[..., 0, P:128] = aih
        acat[..., 1, 0:P] = -aih
        acat[..., 1, P:128] = arh
        siw = np.empty((1, H, 2 * N), np.float32)
        siw[0, :, 0:N] = si_re[b].T
        siw[0, :, N:2 * N] = si_im[b].T
        sjc = np.empty((128, NJC, H, 2), np.float32)
        for jc in range(NJC):
            rows = slice(jh * JH + jc * 128, jh * JH + (jc + 1) * 128)
            sjc[:, jc, :, 0] = sj_re[b, rows, :]
            sjc[:, jc, :, 1] = sj_im[b, rows, :]
        in_maps.append({
            "ltc": ltc.astype(bf),
            "acat": np.ascontiguousarray(acat).astype(bf),
            "siw": siw,
            "sjc": sjc,
            "ones_r": np.ones(128, np.float32),
            "ones_c": np.ones(128, bf),
        })
    return in_maps


def _host_post(results, inputs):
    br = np.asarray(inputs["bias_real"], np.float32)
    bi = np.asarray(inputs["bias_imag"], np.float32)
    out_re = np.empty((B, N, P, H), np.float32)
    out_im = np.empty((B, N, P, H), np.float32)
    for b in range(B):
        y = results[2 * b]["yout"] + results[2 * b + 1]["yout"]  # (H,128,N)
        den = (results[2 * b]["dout"] + results[2 * b + 1]["dout"]).reshape(H, N)
        for h in range(H):
            out_re[b, :, :, h] = (y[h, 0:P] / den[h]).T
            out_im[b, :, :, h] = (y[h, P:128] / den[h]).T
    out_re = out_re.reshape(B, N, ACT_OUT) + br
    out_im = out_im.reshape(B, N, ACT_OUT) + bi
    return out_re, out_im


def _run(inputs, trace=False, **kw):
    from concourse.bass_utils import run_bass_kernel_spmd
    a_re = float(np.asarray(inputs["prelu_a_real"]))
    a_im = float(np.asarray(inputs["prelu_a_imag"]))
    key = ("nc", a_re, a_im)
    if key not in _cache:
        _cache[key] = _build_bass(a_re, a_im)
    nc = _cache[key]
    _cache["nc"] = nc  # for sim_time/trace_tool
    in_maps = _host_prep(inputs)
    res = run_bass_kernel_spmd(nc, in_maps, list(range(8)), trace=trace, **kw)
    out = _host_post(res.results, inputs)
    return out, res


def kernel(**inputs):
    out, _ = _run(inputs, trace=False)
    return out


# revision 38
# speedup vs baseline: 1.1290x; 1.0447x over previous
"""Trainium2 Bass kernel for complex Chebyshev graph conv with attention.

Problem shapes (hardcoded):
  B=4, N=512, C_IN=32, K+1=4 poly terms, H=4 heads, P=64 out/head, ACT=256.

v2 design:
  Sharding: core = 2*b + jh  (jh = j-half). Each core handles ALL 4 heads,
  ALL k, for j in [jh*256, jh*256+256) of graph b — so each core streams
  only HALF of that graph's L (bf16), and the softmax normalization is done
  on the host: Y = (Y_coreA + Y_coreB) / (den_A + den_B).

  Weight folding: host precomputes A[k,h] = X @ W[k,h] (complex, [N,64]).
  With lhsT variants [Ar|Ai] and [-Ai|Ar], the propagation matmuls
  accumulate Yre (psum rows 0:64) and Yim (rows 64:128) directly across
  all (k, jc) — no LX intermediate, no weight matmuls, no sub/add fixups.

  Score path (per head h, j-chunk jc): pre = prelu(si[i] + sj[j]) via ACT
  (si broadcast across partitions via a PE ones-matmul, sj as per-partition
  bias); sq = pre^2; ssum = sq_re + sq_im; E = exp(ssum) in bf16.
  den[h,i] = sum_j E via ones-matmul (partial; host combines).

  Propagation: Mcat[k,ri] = Lt[k,ri] * E (one DVE op per (h,jc) with a
  stride-0-replicated E access pattern), then 8 accumulating matmuls.
"""

import numpy as np

B, N, C = 4, 512, 32
K1, H, P = 4, 4, 64
ACT_OUT = P * H
JH = 256   # j rows per core (half a graph)
NJC = 2    # j chunks of 128 per core

_cache = {}


def _build_bass(alpha_re=0.25, alpha_im=0.25):
    import concourse.bass as bass
    import concourse.mybir as mybir
    import concourse.tile as tile
    from concourse import bacc

    fp32 = mybir.dt.float32
    f32r = mybir.dt.float32r
    bf16 = mybir.dt.bfloat16
    AF = mybir.ActivationFunctionType

    nc = bacc.Bacc("TRN2", target_bir_lowering=False, debug=False)

    ltc = nc.declare_dram_parameter("ltc", [JH, 2 * K1, N], bf16, isOutput=False)
    acat = nc.declare_dram_parameter("acat", [H, JH, K1, 2, 128], bf16,
                                     isOutput=False)
    siw = nc.declare_dram_parameter("siw", [1, H, 2 * N], f32r, isOutput=False)
    sjc = nc.declare_dram_parameter("sjc", [128, NJC, H, 2], fp32, isOutput=False)
    ones_r = nc.declare_dram_parameter("ones_r", [128], f32r, isOutput=False)
    ones_c = nc.declare_dram_parameter("ones_c", [128], bf16, isOutput=False)
    yout = nc.declare_dram_parameter("yout", [H, 128, N], fp32, isOutput=True)
    dout = nc.declare_dram_parameter("dout", [1, H * N], fp32, isOutput=True)

    def rep(sl, n):
        # replicate a [128, F] slice n times along a new middle free dim
        ap = list(sl.ap)
        return bass.AP(tensor=sl.tensor, offset=sl.offset,
                       ap=[ap[0], [0, n]] + ap[1:])

    with tile.TileContext(nc) as tc, nc.allow_low_precision(
            reason="bf16 propagation operands (full-rate PE, 2x DVE)"):
        consts = tc.alloc_tile_pool(name="consts", bufs=1)
        ep = tc.alloc_tile_pool(name="ep", bufs=3)
        sc = tc.alloc_tile_pool(name="sc", bufs=4)
        mp = tc.alloc_tile_pool(name="mp", bufs=3)
        yp = tc.alloc_tile_pool(name="yp", bufs=2)
        pbsi = tc.alloc_tile_pool(name="pbsi", bufs=2, space="PSUM")
        py = tc.alloc_tile_pool(name="py", bufs=2, space="PSUM")
        pden = tc.alloc_tile_pool(name="pden", bufs=2, space="PSUM")
        pools = [consts, ep, sc, mp, yp, pbsi, py, pden]

        # warm the ACT tables needed later so table loads don't land
        # mid-pipeline (memset on Pool: keep DVE free for the small DMAs)
        warm = consts.tile([1, 4], fp32)
        nc.gpsimd.memset(warm, 1.0)
        nc.scalar.activation(warm, warm, AF.Prelu, alpha=alpha_re)
        nc.scalar.activation(warm, warm, AF.Square)
        nc.scalar.activation(warm, warm, AF.Exp)

        # ---- DMA order matters: transfers serialize globally, so the tiny
        # score-path inputs go FIRST on SP, then L/A streams in first-use
        # order. ones_r rides the ACT queue (post-warm), ones_c the SWDGE. ----
        ones_row = consts.tile([1, 128], f32r)
        nc.sync.dma_start(out=ones_row, in_=ones_r[:].rearrange("(o n) -> o n", o=1))
        si_sb = consts.tile([1, H, 2 * N], f32r)
        nc.sync.dma_start(out=si_sb, in_=siw[:])
        sjc_sb = consts.tile([128, NJC, H, 2], fp32)
        nc.sync.dma_start(out=sjc_sb, in_=sjc[:])
        ones_col = consts.tile([128, 1], bf16)
        nc.gpsimd.dma_start(out=ones_col, in_=ones_c[:].rearrange("(n o) -> n o", o=1))

        ltc_sb = consts.tile([128, NJC, 2 * K1, N], bf16)
        acat_sb = consts.tile([128, H, NJC, K1, 2, 128], bf16)

        def acat_dma(h):
            nc.sync.dma_start(
                out=acat_sb[:, h],
                in_=acat[h].rearrange("(c p) k v o -> p c (k v o)", p=128))

        nc.sync.dma_start(out=ltc_sb[:, 0, 0:4, :], in_=ltc[0:128, 0:4, :])
        acat_dma(0)
        nc.sync.dma_start(out=ltc_sb[:, 0, 4:8, :], in_=ltc[0:128, 4:8, :])
        nc.sync.dma_start(out=ltc_sb[:, 1, 0:4, :], in_=ltc[128:256, 0:4, :])
        nc.sync.dma_start(out=ltc_sb[:, 1, 4:8, :], in_=ltc[128:256, 4:8, :])
        acat_dma(1)
        acat_dma(2)
        acat_dma(3)

        E = [None] * H
        bsi = [None] * H
        den_sb = consts.tile([1, H * N], fp32)

        def emit_bsi(h):
            # si broadcast across partitions via PE; prelu reads it from PSUM
            bps = pbsi.tile([128, 2 * N], fp32, tag="bsi")
            bsi[h] = bps
            nc.tensor.matmul(bps[:, 0:N], ones_row, si_sb[:, h, 0:N],
                             start=True, stop=True)
            nc.tensor.matmul(bps[:, N:2 * N], ones_row, si_sb[:, h, N:2 * N],
                             start=True, stop=True)

        f16 = mybir.dt.float16

        def emit_score_front(h, fast=False):
            # prelus + jc0 square in fp16 (2x DVE on squares; exp's input
            # error ~1e-3 relative stays well under the gate)
            E[h] = ep.tile([128, NJC, N], bf16, tag="E", name=f"E{h}")
            pres = []
            for jc in range(NJC):
                pre = sc.tile([128, 2 * N], f16, tag="pre")
                nc.scalar.activation(pre[:, 0:N], bsi[h][:, 0:N], AF.Prelu,
                                     bias=sjc_sb[:, jc, h, 0:1], alpha=alpha_re)
                nc.scalar.activation(pre[:, N:2 * N], bsi[h][:, N:2 * N],
                                     AF.Prelu, bias=sjc_sb[:, jc, h, 1:2],
                                     alpha=alpha_im)
                pres.append(pre)
            sq0 = sc.tile([128, 2 * N], f16, tag="sq0")
            if h % 2 == 0:
                nc.scalar.activation(sq0, pres[0], AF.Square)
            else:
                nc.vector.tensor_mul(sq0, pres[0], pres[0])
            return [sq0, pres[1]]

        def emit_score_back(h, sqs, fast=False):
            # jc1 square + both adds on DVE (fp16 2x), emitted after the
            # current head's mega-muls so DVE never stalls waiting prelus
            sq0, pre1 = sqs
            sq1 = sc.tile([128, 2 * N], f16, tag="sq1")
            nc.vector.tensor_mul(sq1, pre1, pre1)
            for jc, sq in ((0, sq0), (1, sq1)):
                ssum = sc.tile([128, N], f16, tag="ssum")
                nc.vector.tensor_add(ssum, sq[:, 0:N], sq[:, N:2 * N])
                nc.scalar.activation(E[h][:, jc, :], ssum, AF.Exp)

        def emit_den(h):
            dps = pden.tile([1, N], fp32, tag="den")
            for jc in range(NJC):
                nc.tensor.matmul(dps, ones_col, E[h][:, jc, :],
                                 start=(jc == 0), stop=(jc == NJC - 1))
            if h % 2 == 0:
                nc.vector.tensor_copy(den_sb[:, h * N:(h + 1) * N], dps)
            else:
                nc.scalar.copy(den_sb[:, h * N:(h + 1) * N], dps)

        ps_y = [None] * H

        def emit_kloop(h, pool_share=True):
            # per jc: k0-1 rows on DVE first (so PE can start), then k2 on
            # DVE and the k3 pair on Pool; matmuls interleave per piece
            ps = py.tile([128, N], fp32, tag="ps_y")
            ps_y[h] = ps

            def mms(jc, mcat, k0, k1):
                for k in range(k0, k1):
                    nc.tensor.matmul(ps, acat_sb[:, h, jc, k, 0],
                                     mcat[:, 2 * k, :],
                                     start=(jc == 0 and k == 0), stop=False)
                    nc.tensor.matmul(ps, acat_sb[:, h, jc, k, 1],
                                     mcat[:, 2 * k + 1, :],
                                     start=False,
                                     stop=(jc == NJC - 1 and k == K1 - 1))

            for jc in range(NJC):
                mcat = mp.tile([128, 2 * K1, N], bf16, tag="mcat")
                erow = E[h][:, jc, :]
                nc.vector.tensor_mul(mcat[:, 0:4, :], ltc_sb[:, jc, 0:4, :],
                                     rep(erow, 4))
                if pool_share:
                    nc.gpsimd.tensor_mul(mcat[:, 6:8, :],
                                         ltc_sb[:, jc, 6:8, :], rep(erow, 2))
                    nc.vector.tensor_mul(mcat[:, 4:6, :],
                                         ltc_sb[:, jc, 4:6, :], rep(erow, 2))
                    mms(jc, mcat, 0, 2)
                    mms(jc, mcat, 2, 4)
                else:
                    nc.vector.tensor_mul(mcat[:, 4:8, :],
                                         ltc_sb[:, jc, 4:8, :], rep(erow, 4))
                    mms(jc, mcat, 0, 2)
                    mms(jc, mcat, 2, 4)

        dmaq = [nc.sync, nc.scalar, nc.scalar, nc.sync]

        def emit_out(h):
            y_sb = yp.tile([128, N], fp32, tag="y_sb")
            if h == H - 1:
                # last head: split the copy ACT/DVE so the tail is short
                nc.scalar.copy(y_sb[:, 0:N // 2], ps_y[h][:, 0:N // 2])
                nc.vector.tensor_copy(y_sb[:, N // 2:N], ps_y[h][:, N // 2:N])
            elif h % 2 == 0:
                nc.scalar.copy(y_sb, ps_y[h])
            else:
                nc.vector.tensor_copy(y_sb, ps_y[h])
            dmaq[h].dma_start(out=yout[h], in_=y_sb)

        # ---- pipelined emission: 2-deep score lookahead ----
        emit_bsi(0)
        sq_0 = emit_score_front(0, fast=True)
        emit_score_back(0, sq_0, fast=True)
        emit_bsi(1)
        sq_1 = emit_score_front(1)
        emit_score_back(1, sq_1)

        sqs = {}
        for h in range(H):
            if h + 2 < H:
                emit_bsi(h + 2)
                sqs[h + 2] = emit_score_front(h + 2)
            emit_den(h)
            emit_kloop(h, pool_share=(h > 0))
            if h + 2 < H:
                emit_score_back(h + 2, sqs[h + 2])
            emit_out(h)
        nc.sync.dma_start(out=dout[:], in_=den_sb)

        for p_ in reversed(pools):
            p_.release()

    nc.compile()
    return nc


def _host_prep(inputs):
    """Build the 8 per-core input maps from the full inputs."""
    import ml_dtypes
    bf = ml_dtypes.bfloat16
    Xr = np.asarray(inputs["X_real"], np.float32)
    Xi = np.asarray(inputs["X_imag"], np.float32)
    Lr = np.asarray(inputs["L_real"], np.float32)
    Li = np.asarray(inputs["L_imag"], np.float32)
    awr = np.asarray(inputs["attn_w_real"], np.float32)
    awi = np.asarray(inputs["attn_w_imag"], np.float32)
    abr = np.asarray(inputs["attn_b_real"], np.float32)
    abi = np.asarray(inputs["attn_b_imag"], np.float32)
    wr = np.asarray(inputs["weight_real"], np.float32)
    wi = np.asarray(inputs["weight_imag"], np.float32)

    W1r, W2r = awr[:C], awr[C:]
    W1i, W2i = awi[:C], awi[C:]
    si_re = Xr @ W1r - Xi @ W1i + abr  # (B,N,H) (+bias folded)
    si_im = Xr @ W1i + Xi @ W1r + abi
    sj_re = Xr @ W2r - Xi @ W2i
    sj_im = Xr @ W2i + Xi @ W2r

    LTr = Lr.swapaxes(-1, -2)  # (B,K1,j,i)
    LTi = Li.swapaxes(-1, -2)

    # A[b,k,j,p,h] = complex X @ W per head
    Wr4 = wr.reshape(K1, C, P, H)
    Wi4 = wi.reshape(K1, C, P, H)
    Ar = (np.einsum('bjc,kcph->bkjph', Xr, Wr4)
          - np.einsum('bjc,kcph->bkjph', Xi, Wi4))
    Ai = (np.einsum('bjc,kcph->bkjph', Xi, Wr4)
          + np.einsum('bjc,kcph->bkjph', Xr, Wi4))

    in_maps = []
    for core in range(8):
        b, jh = core // 2, core % 2
        js = slice(jh * JH, (jh + 1) * JH)
        ltc = np.empty((JH, 2 * K1, N), np.float32)
        ltc[:, 0::2, :] = LTr[b, :, js, :].swapaxes(0, 1)
        ltc[:, 1::2, :] = LTi[b, :, js, :].swapaxes(0, 1)
        # acat[h,j,k,0] = [Ar|Ai], acat[h,j,k,1] = [-Ai|Ar]
        acat = np.empty((H, JH, K1, 2, 128), np.float32)
        arh = Ar[b, :, js].transpose(3, 1, 0, 2)  # (h,j,k,p)
        aih = Ai[b, :, js].transpose(3, 1, 0, 2)
        acat[..., 0, 0:P] = arh
        acat[..., 0, P:128] = aih
        acat[..., 1, 0:P] = -aih
        acat[..., 1, P:128] = arh
        siw = np.empty((1, H, 2 * N), np.float32)
        siw[0, :, 0:N] = si_re[b].T
        siw[0, :, N:2 * N] = si_im[b].T
        sjc = np.empty((128, NJC, H, 2), np.float32)
        for jc in range(NJC):
            rows = slice(jh * JH + jc * 128, jh * JH + (jc + 1) * 128)
            sjc[:, jc, :, 0] = sj_re[b, rows, :]
            sjc[:, jc, :, 1] = sj_im[b, rows, :]
        in_maps.append({
            "ltc": ltc.astype(bf),
            "acat": np.ascontiguousarray(acat).astype(bf),
            "siw": siw,
            "sjc": sjc,
            "ones_r": np.ones(128, np.float32),
            "ones_c": np.ones(128, bf),
        })
    return in_maps


def _host_post(results, inputs):
    br = np.asarray(inputs["bias_real"], np.float32)
    bi = np.asarray(inputs["bias_imag"], np.float32)
    out_re = np.empty((B, N, P, H), np.float32)
    out_im = np.empty((B, N, P, H), np.float32)
    for b in range(B):
        y = results[2 * b]["yout"] + results[2 * b + 1]["yout"]  # (H,128,N)
        den = (results[2 * b]["dout"] + results[2 * b + 1]["dout"]).reshape(H, N)
        for h in range(H):
            out_re[b, :, :, h] = (y[h, 0:P] / den[h]).T
            out_im[b, :, :, h] = (y[h, P:128] / den[h]).T
    out_re = out_re.reshape(B, N, ACT_OUT) + br
    out_im = out_im.reshape(B, N, ACT_OUT) + bi
    return out_re, out_im


def _run(inputs, trace=False, **kw):
    from concourse.bass_utils import run_bass_kernel_spmd
    a_re = float(np.asarray(inputs["prelu_a_real"]))
    a_im = float(np.asarray(inputs["prelu_a_imag"]))
    key = ("nc", a_re, a_im)
    if key not in _cache:
        _cache[key] = _build_bass(a_re, a_im)
    nc = _cache[key]
    _cache["nc"] = nc  # for sim_time/trace_tool
    in_maps = _host_prep(inputs)
    res = run_bass_kernel_spmd(nc, in_maps, list(range(8)), trace=trace, **kw)
    out = _host_post(res.results, inputs)
    return out, res


def kernel(**inputs):
    out, _ = _run(inputs, trace=False)
    return out
